# revision 7
# baseline (speedup 1.0000x reference)
"""Trainium2 Bass kernel for nn_Block_65755949302136 (dense transformer block).

Sharding: 8 cores = 2 (batch) x 4 (tensor-parallel ranks). Rank r owns heads
[r, 4+r, 8+2r, 9+2r] (slot0 = strongly-sloped ALiBi head with a 3-block
causal window, slot1 = weakly-sloped full-causal head, slots 2/3 zero-slope),
the matching w_in column slices and w_out row slice. ReduceScatter(add) over
each batch group after out_proj, LN2 on each rank's 512-row shard.

v2 design:
- LN1 gamma, per-head q/k scales and ln1_beta column corrections are folded
  into the weights on the host.
- LN1 stats are broadcast [128,512] matmuls (ones/D stationary) so the
  var/rsqrt chain runs partition-parallel; x is centered+scaled in place
  (xn = (x-mu)*rstd), removing all extended-contraction matmuls.
- x is loaded chunk-major (4 DMAs of [128, 8x512]) so stats/xn/v/qkvp
  pipeline per 512-token chunk; weights load as one DMA per kind.
- Softmax denominator rides the AV matmul: per 128-query tile the stationary
  is the exp tile and the moving operand is [v | ones] (129 cols), giving a
  token-major o plus its denominator column in one pass; the normalized,
  beta-corrected o is transposed back to feature-major on the PE and gated
  into silu(p).
- Analytic per-(q-tile,k-block) shift rides the ACT exp bias.
"""

import sys

sys.path.insert(0, "/opt/trn_rl_repo")

from collections import deque

import numpy as np

import concourse.bass as bass
import concourse.mybir as mybir
import concourse.tile as tile
from concourse.bass_utils import run_bass_kernel_spmd

F32 = mybir.dt.float32
BF16 = mybir.dt.bfloat16
NP_BF16 = mybir.dt.np(BF16)
AF = mybir.ActivationFunctionType
ALU = mybir.AluOpType

B, L, D, NHEADS, DH = 2, 2048, 1024, 16, 128
DEXP = 2048  # full d_expanded
NH = 4  # heads per core
DL = NH * DH  # 512, local d_expanded slice
KT = D // 128  # 8 k-tiles over d_model
NCH = L // 512  # 4 query chunks
NMT = L // 128  # 16 token tiles
NG = 4  # reduce-scatter groups (512 rows each)

# per-slot causal block window (slot0 = heads 0-3, min slope 0.0928 -> 3 blocks)
WB = {0: 2, 1: 16, 2: 16, 3: 16}

_CACHED = {}


def _normalize_waits(nc):
    """walrus wait-slot limits are tighter than what Tile emits for some
    instruction classes; move excess sync-waits onto same-engine NoOp
    carriers inserted immediately before the instruction."""
    for func in nc.m.functions:
        for blk in func.blocks:
            insts = blk.instructions
            i = 0
            while i < len(insts):
                inst = insts[i]
                si = inst.sync_info
                cap = 1
                if si is not None and len(si.on_wait or []) > cap:
                    waits = list(si.on_wait)
                    excess, keep = waits[:-cap], waits[-cap:]
                    for j, w in enumerate(excess):
                        d = mybir.InstNoOp(
                            name=f"{inst.name}-wsplit{j}",
                            engine=inst.engine,
                            ins=[],
                            outs=[],
                        )
                        d.sync_info = mybir.SyncInfo(on_wait=[w], on_update=[])
                        insts.insert(i, d)
                        nc.register_instruction(d, overwrite=True)
                        i += 1
                    si.on_wait = keep
                i += 1


def build(with_cc=True):
    nc = bass.Bass()

    xt_d = nc.dram_tensor("xt", [D, L], BF16, kind="ExternalInput")
    wq_d = nc.dram_tensor("wq", [D, DL], BF16, kind="ExternalInput")
    wk_d = nc.dram_tensor("wk", [D, DL], BF16, kind="ExternalInput")
    wv_d = nc.dram_tensor("wv", [D, DL], BF16, kind="ExternalInput")
    wp_d = nc.dram_tensor("wp", [D, DL], BF16, kind="ExternalInput")
    wout_d = nc.dram_tensor("wout", [DL, D], BF16, kind="ExternalInput")
    smallf_d = nc.dram_tensor("smallf", [128, 35], F32, kind="ExternalInput")
    smallb_d = nc.dram_tensor("smallb", [128, 256 + DL], BF16, kind="ExternalInput")
    g2b2_d = nc.dram_tensor("g2b2", [128, 2 * D], BF16, kind="ExternalInput")
    out_d = nc.dram_tensor("out", [NG * 128, D], BF16, kind="ExternalOutput")

    with tile.TileContext(nc, pool_alloc_mode="queue") as tc:
        cp_cm = tc.tile_pool(name="const", bufs=1)
        cp = cp_cm.__enter__()
        xbp_cm = tc.tile_pool(name="xbp", bufs=1)
        xbp = xbp_cm.__enter__()
        wp_cm = tc.tile_pool(name="wpool", bufs=1)
        wpo = wp_cm.__enter__()
        wop_cm = tc.tile_pool(name="wo", bufs=1)
        wop = wop_cm.__enter__()
        dram_cm = tc.tile_pool(name="dram", bufs=1, space="DRAM")
        dram = dram_cm.__enter__()

        # ---- big DMAs, ordered by first use ----
        xch = []
        xch3 = []
        w_t = {}
        w3 = {}

        def dma_x(ch, split=1):
            t = xbp.tile([128, KT * 512], BF16, tag="x", bufs=NCH, name=f"xch{ch}")
            csl = slice(ch * 512, (ch + 1) * 512)
            t3 = t[:, :].rearrange("p (a n) -> p a n", n=512)
            s3 = xt_d[:, csl].rearrange("(a p) n -> p a n", p=128)
            step = KT // split
            for i in range(split):
                nc.sync.dma_start(
                    t3[:, i * step : (i + 1) * step, :],
                    s3[:, i * step : (i + 1) * step, :],
                )
            xch.append(t)
            xch3.append(t3)

        def dma_w(kind, wd):
            t = wpo.tile([128, KT * DL], BF16, tag="w", bufs=4, name=f"w{kind}")
            nc.sync.dma_start(
                t[:, :].rearrange("p (a n) -> p a n", n=DL),
                wd[:, :].rearrange("(a p) n -> p a n", p=128),
            )
            w_t[kind] = t
            w3[kind] = t[:, :].rearrange("p (a n) -> p a n", n=DL)

        dma_x(0, split=4)
        dma_w("v", wv_d)
        dma_x(1, split=2)
        dma_w("q", wq_d)
        dma_w("k", wk_d)
        dma_w("p", wp_d)
        dma_x(2, split=2)
        dma_x(3, split=2)
        smallb = cp.tile([128, 256 + DL], BF16, tag="smallb")
        nc.sync.dma_start(smallb[:], smallb_d[:, :])
        tri = smallb[:, 0:128]
        iden = smallb[:, 128:256]
        cvbc = smallb[:, 256 : 256 + DL]
        smallf = cp.tile([128, 35], F32, tag="smallf")
        nc.sync.dma_start(smallf[:], smallf_d[:, :])
        ccols = smallf[:, 0:12]
        ratio = smallf[:, 12:16]
        biasv = smallf[:, 16:35]


        woutT = wop.tile([128, NH * D], BF16, tag="woutT")
        nc.sync.dma_start(
            woutT[:, :].rearrange("p (a n) -> p a n", n=D),
            wout_d[:, :].rearrange("(a p) n -> p a n", p=128),
        )
        wo3 = woutT[:, :].rearrange("p (a n) -> p a n", n=D)
        g2b2 = wop.tile([128, 2 * D], BF16, tag="g2b2")
        nc.sync.dma_start(g2b2[:], g2b2_d[:, :])
        g2bc = g2b2[:, 0:D]
        b2bc = g2b2[:, D : 2 * D]

        onesD = cp.tile([128, 128], BF16, tag="onesD")
        nc.gpsimd.memset(onesD[:], 1.0 / D)
        eps128 = cp.tile([128, 1], F32, tag="eps128")
        nc.gpsimd.memset(eps128[:], 1e-5)
        warmup_n = 15

        # ---- main pools ----
        qkp_cm = tc.tile_pool(name="qkp", bufs=1)
        qkp = qkp_cm.__enter__()
        gp_cm = tc.tile_pool(name="gp", bufs=1)
        gp = gp_cm.__enter__()
        vmp_cm = tc.tile_pool(name="vmp", bufs=1)
        vmp = vmp_cm.__enter__()
        etp_cm = tc.tile_pool(name="etp", bufs=30)
        etp = etp_cm.__enter__()
        onp_cm = tc.tile_pool(name="onp", bufs=8)
        onp = onp_cm.__enter__()
        lnp_cm = tc.tile_pool(name="ln2", bufs=1)
        lnp = lnp_cm.__enter__()
        osp_cm = tc.tile_pool(name="ostage", bufs=4)
        osp = osp_cm.__enter__()

        # psum pool for main matmuls; coexists with the stage-1 stats pool
        pmm_cm = tc.tile_pool(name="ps_mm", bufs=2, space="PSUM")
        pmm = pmm_cm.__enter__()
        # pss/po/pot are created after the stats pools close (assigned below,
        # before first use; the emitters close over these names)
        pss = po = pot = None
        pstat = xsqp = scr = srp = None

        def emit_stats(ch):
            mu_ps = pstat.tile([128, 512], F32, tag="mu", name=f"mu{ch}")
            ms_ps = pstat.tile([128, 512], F32, tag="ms", name=f"ms{ch}")
            for kt in range(KT):
                nc.tensor.matmul(
                    mu_ps[:], onesD[:], xch3[ch][:, kt, :],
                    start=(kt == 0), stop=(kt == KT - 1),
                )
            for quar in range(4):
                xsq = xsqp.tile(
                    [128, 2 * 512], BF16, tag="xsq", bufs=3, name=f"xsq{ch}_{quar}"
                )
                xsq3 = xsq[:, :].rearrange("p (a n) -> p a n", n=512)
                nc.scalar.activation(
                    xsq3[:, :, :], xch3[ch][:, 2 * quar : 2 * quar + 2, :], AF.Square
                )
                for kt in range(2):
                    nc.tensor.matmul(
                        ms_ps[:], onesD[:], xsq3[:, kt, :],
                        start=(quar == 0 and kt == 0), stop=(quar == 3 and kt == 1),
                    )
            t1 = scr.tile([128, 512], F32, tag="t1")
            nc.scalar.activation(t1[:], mu_ps[:], AF.Square)
            nc.vector.tensor_sub(t1[:], ms_ps[:], t1[:])
            nc.scalar.activation(t1[:], t1[:], AF.Sqrt, bias=eps128[:])
            rs_bc = srp.tile([128, 512], BF16, tag="rs_bc", name=f"rs{ch}")
            with nc.allow_low_precision("bf16 rstd feeds bf16 muls"):
                nc.vector.reciprocal(rs_bc[:], t1[:])
            mu_bc = srp.tile([128, 512], BF16, tag="mu_bc", name=f"mubc{ch}")
            nc.scalar.copy(mu_bc[:], mu_ps[:])
            mu_b = mu_bc[:, :].rearrange("p (a n) -> p a n", a=1).broadcast_to(
                (128, KT, 512)
            )
            rs_b = rs_bc[:, :].rearrange("p (a n) -> p a n", a=1).broadcast_to(
                (128, KT, 512)
            )
            nc.vector.tensor_sub(xch3[ch][:], xch3[ch][:], mu_b)
            nc.vector.tensor_mul(xch3[ch][:], xch3[ch][:], rs_b)

        qT = [None] * NH
        kS = [None] * NH
        geff = [None] * NH
        _kk = [None] * NH
        vm3 = [None] * NMT
        rs_in = [dram.tile([512, D], BF16, tag=f"rsin{g}", name=f"rsin{g}") for g in range(NG)]
        rs_out = [dram.tile([128, D], BF16, tag=f"rsout{g}", name=f"rsout{g}") for g in range(NG)]

        # deferral machinery
        gqueue = deque()
        carry = []  # pending per-qt normalization closures

        def flush_carry():
            while carry:
                carry.pop(0)()

        def pop1():
            gqueue.popleft()()

        def emit_v_ch(ch):
            for mi in range(4):
                m = 4 * ch + mi
                msl = slice(mi * 128, (mi + 1) * 128)
                vps = pmm.tile([128, 512], F32, tag="mm", name=f"vps{m}")
                for kt in range(KT):
                    nc.tensor.matmul(
                        vps[:], xch3[ch][:, kt, msl], w3["v"][:, kt, :],
                        start=(kt == 0), stop=(kt == KT - 1),
                    )
                t = vmp.tile([128, NH * 129], BF16, tag="vm", bufs=NMT, name=f"vm{m}")
                t3 = t[:, :].rearrange("p (a b) -> p a b", b=129)
                nc.gpsimd.memset(t3[:, :, 128:129], 1.0)
                nc.scalar.copy(t3[:, :, 0:128], vps[:])
                vm3[m] = t3

        def emit_qkvp_chunk(h, ch):
            hsl = slice(h * 128, (h + 1) * 128)
            csl = slice(ch * 512, (ch + 1) * 512)
            if ch == 0:
                qT[h] = qkp.tile([128, L], BF16, tag="qT", bufs=3, name=f"qT{h}")
                kS[h] = qkp.tile([128, L], BF16, tag="kS", bufs=3, name=f"kS{h}")
                geff[h] = gp.tile([128, L], BF16, tag="geff", bufs=NH, name=f"geff{h}")
                _kk[h] = qkp.tile([128, L], BF16, tag="kk", bufs=2, name=f"kk{h}")
            kk = _kk[h]
            qps = pmm.tile([128, 512], F32, tag="mm", name=f"qps{h}_{ch}")
            for kt in range(KT):
                nc.tensor.matmul(qps[:], w3["q"][:, kt, hsl], xch3[ch][:, kt, :],
                                 start=(kt == 0), stop=(kt == KT - 1))
            nc.vector.tensor_scalar_add(qT[h][:, csl], qps[:], ccols[:, h : h + 1])
            kps = pmm.tile([128, 512], F32, tag="mm", name=f"kps{h}_{ch}")
            for kt in range(KT):
                nc.tensor.matmul(kps[:], w3["k"][:, kt, hsl], xch3[ch][:, kt, :],
                                 start=(kt == 0), stop=(kt == KT - 1))
            nc.vector.tensor_scalar_add(kk[:, csl], kps[:], ccols[:, NH + h : NH + h + 1])
            pps = pmm.tile([128, 512], F32, tag="mm", name=f"pps{h}_{ch}")
            for kt in range(KT):
                nc.tensor.matmul(pps[:], w3["p"][:, kt, hsl], xch3[ch][:, kt, :],
                                 start=(kt == 0), stop=(kt == KT - 1))
            nc.scalar.activation(
                geff[h][:, csl], pps[:], AF.Silu,
                bias=ccols[:, 2 * NH + h : 2 * NH + h + 1],
            )

        def emit_smear(h):
            kk = _kk[h]
            nc.vector.scalar_tensor_tensor(
                kS[h][:, 1:L], kk[:, 0 : L - 1], ratio[:, h : h + 1], kk[:, 1:L],
                ALU.mult, ALU.add,
            )
            nc.vector.tensor_copy(kS[h][:, 0:1], kk[:, 0:1])

        et_info = {}

        def emit_S(h, ch):
            wb = WB[h]
            csl = slice(ch * 512, (ch + 1) * 512)
            kb_lo = max(0, 4 * ch + 1 - wb)
            kb_hi = 4 * ch + 3
            info = {}
            for kb in range(kb_lo, kb_hi + 1):
                qs0 = max(0, kb - 4 * ch)
                qs1 = min(4, kb - 4 * ch + wb)
                if qs0 >= qs1:
                    continue
                w = (qs1 - qs0) * 128
                nsl = slice(csl.start + qs0 * 128, csl.start + qs1 * 128)
                sps = pss.tile([128, w], F32, tag="sps", name=f"sps{h}_{ch}_{kb}")
                nc.tensor.matmul(
                    sps[:], kS[h][:, kb * 128 : (kb + 1) * 128], qT[h][:, nsl],
                    start=True, stop=True,
                )
                et = etp.tile([128, w], BF16, tag="et", name=f"et{h}_{ch}_{kb}")
                if h == 0:
                    for qs in range(qs0, qs1):
                        esl = slice((qs - qs0) * 128, (qs - qs0 + 1) * 128)
                        dd = (4 * ch + qs) - kb
                        nc.scalar.activation(
                            et[:, esl], sps[:, esl], AF.Exp,
                            bias=biasv[:, dd : dd + 1],
                        )
                elif h == 1:
                    dd = 4 * ch - kb + 3  # in [0, 15]
                    nc.scalar.activation(
                        et[:, :], sps[:, :], AF.Exp, bias=biasv[:, 3 + dd : 4 + dd]
                    )
                else:
                    nc.scalar.activation(et[:, :], sps[:, :], AF.Exp)
                qs_diag = kb - 4 * ch
                if qs0 <= qs_diag < qs1:
                    esl = slice((qs_diag - qs0) * 128, (qs_diag - qs0 + 1) * 128)
                    nc.gpsimd.tensor_mul(et[:, esl], et[:, esl], tri[:])
                info[kb] = (et, qs0)
            et_info[(h, ch)] = info

        def make_norm(h, qt, o):
            def norm():
                dinv = onp.tile([128, 1], F32, tag="dinv", name=f"dinv{h}_{qt}")
                nc.vector.reciprocal(dinv[:], o[:, 128:129])
                onrm = onp.tile([128, 128], BF16, tag="onrm", name=f"onrm{h}_{qt}")
                nc.vector.scalar_tensor_tensor(
                    onrm[:], o[:, 0:128], dinv[:],
                    cvbc[:, h * 128 : (h + 1) * 128], ALU.mult, ALU.add,
                )
                ot = pot.tile([128, 128], BF16, tag="ot", name=f"ot{h}_{qt}")
                nc.tensor.transpose(ot[:], onrm[:], iden[:])
                msl = slice(qt * 128, (qt + 1) * 128)
                nc.vector.tensor_mul(geff[h][:, msl], ot[:], geff[h][:, msl])

            return norm

        def make_AV(h, ch):
            def av():
                flush_carry()
                wb = WB[h]
                info = et_info.pop((h, ch))
                for qs in range(4):
                    qt = 4 * ch + qs
                    kbs = list(range(max(0, qt - wb + 1), qt + 1))
                    o = po.tile([128, 129], F32, tag="o", name=f"o{h}_{qt}")
                    for kb in kbs:
                        et, qs0 = info[kb]
                        esl = slice((qs - qs0) * 128, (qs - qs0 + 1) * 128)
                        nc.tensor.matmul(
                            o[:], et[:, esl], vm3[kb][:, h, :],
                            start=(kb == kbs[0]), stop=(kb == kbs[-1]),
                        )
                    carry.append(make_norm(h, qt, o))
                    if len(carry) >= 3:
                        carry.pop(0)()

            return av

        def emit_op(g):
            flush_carry()
            for mi in range(4):
                m = 4 * g + mi
                msl = slice(m * 128, (m + 1) * 128)
                for n2 in range(2):
                    nsl2 = slice(n2 * 512, (n2 + 1) * 512)
                    op2 = pmm.tile([128, 512], F32, tag="mm", name=f"op2_{m}_{n2}")
                    for hh in range(NH):
                        nc.tensor.matmul(
                            op2[:], geff[hh][:, msl], wo3[:, hh, nsl2],
                            start=(hh == 0), stop=(hh == NH - 1),
                        )
                    osb = osp.tile([128, 512], BF16, tag="osb")
                    if g == NG - 1:
                        nc.scalar.copy(osb[:], op2[:])
                    else:
                        nc.vector.tensor_copy(osb[:], op2[:])
                    nc.sync.dma_start(rs_in[g][mi * 128 : (mi + 1) * 128, nsl2], osb[:])
            if with_cc:
                nc.gpsimd.collective_compute(
                    "ReduceScatter", ALU.add,
                    replica_groups=[[0, 1, 2, 3], [4, 5, 6, 7]],
                    ins=[rs_in[g][:, :].opt()],
                    outs=[rs_out[g][:, :].opt()],
                )
            else:
                nc.sync.dma_start(rs_out[g][:, :], rs_in[g][0:128, :])
            yt = lnp.tile([128, D], BF16, tag="yt")
            bs = lnp.tile([128, 12], F32, tag="bs")
            for half in range(2):
                hsl2 = slice(half * 512, (half + 1) * 512)
                nc.sync.dma_start(yt[:, hsl2], rs_out[g][:, hsl2])
                nc.vector.bn_stats(bs[:, 6 * half : 6 * half + 6], yt[:, hsl2])
            ag = lnp.tile([128, 2], F32, tag="ag")
            nc.vector.bn_aggr(ag[:], bs[:])
            sd2 = lnp.tile([128, 1], F32, tag="sd2")
            nc.scalar.activation(sd2[:], ag[:, 1:2], AF.Sqrt, bias=eps128[:])
            rstd2 = lnp.tile([128, 1], F32, tag="rstd2")
            nc.vector.reciprocal(rstd2[:], sd2[:])
            nmu = lnp.tile([128, 1], F32, tag="nmu")
            nc.vector.tensor_scalar_mul(nmu[:], ag[:, 0:1], -1.0)
            t2 = lnp.tile([128, D], BF16, tag="t2")
            for half in range(2):
                hsl2 = slice(half * 512, (half + 1) * 512)
                nc.vector.tensor_scalar(
                    t2[:, hsl2], yt[:, hsl2], nmu[:], rstd2[:], ALU.add, ALU.mult
                )
                nc.vector.tensor_mul(t2[:, hsl2], t2[:, hsl2], g2bc[:, hsl2])
                nc.vector.tensor_add(t2[:, hsl2], t2[:, hsl2], b2bc[:, hsl2])
            nc.sync.dma_start(out_d[g * 128 : (g + 1) * 128, :], t2[:])

        # ---- emission schedule ----
        st_cms = [
            tc.tile_pool(name="ps_stat", bufs=2, space="PSUM"),
            tc.tile_pool(name="xsqp", bufs=1),
            tc.tile_pool(name="scr", bufs=1),
            tc.tile_pool(name="srow", bufs=2),
        ]
        pstat, xsqp, scr, srp = [cm.__enter__() for cm in st_cms]
        for wi in range(warmup_n):
            wps = pmm.tile([128, 128], F32, tag="mm", name=f"warm{wi}")
            nc.tensor.matmul(wps[:], onesD[:], onesD[:], start=True, stop=True)
        emit_stats(0)
        emit_stats(1)
        emit_v_ch(0)
        emit_qkvp_chunk(0, 0)
        emit_qkvp_chunk(1, 0)
        emit_stats(2)
        emit_v_ch(1)
        emit_qkvp_chunk(0, 1)
        emit_qkvp_chunk(1, 1)
        emit_stats(3)
        emit_v_ch(2)
        emit_qkvp_chunk(0, 2)
        emit_qkvp_chunk(1, 2)
        emit_v_ch(3)
        emit_qkvp_chunk(0, 3)
        emit_qkvp_chunk(1, 3)
        for cm in reversed(st_cms):
            cm.__exit__(None, None, None)
        emit_smear(0)
        emit_smear(1)
        pss_cm = tc.tile_pool(name="ps_s", bufs=3, space="PSUM")
        pss = pss_cm.__enter__()
        po_cm = tc.tile_pool(name="ps_o", bufs=2, space="PSUM")
        po = po_cm.__enter__()
        pot_cm = tc.tile_pool(name="ps_ot", bufs=1, space="PSUM")
        pot = pot_cm.__enter__()
        # E1: attn0 interleaved with qkvp2
        for ch in range(NCH):
            emit_S(0, ch)
            gqueue.append(make_AV(0, ch))
            emit_qkvp_chunk(2, ch)
            if len(gqueue) > 1:
                pop1()
        emit_smear(2)
        # E2: attn1 interleaved with qkvp3 (defer depth 2)
        for ch in range(NCH):
            emit_S(1, ch)
            gqueue.append(make_AV(1, ch))
            emit_qkvp_chunk(3, ch)
            if len(gqueue) > 2:
                pop1()
        emit_smear(3)
        # E3: attn2 + attn3 + out_proj, pipelined
        for ch in range(NCH):
            emit_S(2, ch)
            gqueue.append(make_AV(2, ch))
            pop1()
            if ch >= 2:
                emit_op(ch - 2)
            emit_S(3, ch)
            gqueue.append(make_AV(3, ch))
            pop1()
        pop1()
        emit_op(NG - 2)
        pop1()
        assert not gqueue
        emit_op(NG - 1)
        flush_carry()

        pot_cm.__exit__(None, None, None)
        po_cm.__exit__(None, None, None)
        pss_cm.__exit__(None, None, None)
        pmm_cm.__exit__(None, None, None)
        osp_cm.__exit__(None, None, None)
        lnp_cm.__exit__(None, None, None)
        onp_cm.__exit__(None, None, None)
        etp_cm.__exit__(None, None, None)
        vmp_cm.__exit__(None, None, None)
        gp_cm.__exit__(None, None, None)
        qkp_cm.__exit__(None, None, None)
        dram_cm.__exit__(None, None, None)
        wop_cm.__exit__(None, None, None)
        wp_cm.__exit__(None, None, None)
        xbp_cm.__exit__(None, None, None)
        cp_cm.__exit__(None, None, None)

    _normalize_waits(nc)
    return nc


def _slopes16():
    half = NHEADS // 2
    return np.concatenate(
        [2.0 ** np.linspace(0.0, -8.0, half), np.zeros(NHEADS - half)]
    ).astype(np.float32)


def kernel(x, ln1_g, ln1_b, ln2_g, ln2_b, w_in, w_out, smear_factor, log_scale):
    x = np.asarray(x, np.float32)
    w_in = np.asarray(w_in, np.float32)
    w_out = np.asarray(w_out, np.float32)
    ln1_g = np.asarray(ln1_g, np.float32)
    ln1_b = np.asarray(ln1_b, np.float32)
    ln2_g = np.asarray(ln2_g, np.float32)
    ln2_b = np.asarray(ln2_b, np.float32)
    smear_factor = np.asarray(smear_factor, np.float32)
    log_scale = np.asarray(log_scale, np.float32)

    if "nc" not in _CACHED:
        _CACHED["nc"] = build()
    nc = _CACHED["nc"]

    slopes16 = _slopes16()
    jj = np.arange(128)
    tri = (jj[:, None] <= jj[None, :]).astype(NP_BF16)  # keep j <= i
    iden = np.eye(128, dtype=NP_BF16)
    iota = np.arange(128, dtype=np.float32)

    in_maps = []
    for c in range(8):
        b, r = divmod(c, 4)
        hs = [r, 4 + r, 8 + 2 * r, 9 + 2 * r]
        cols = np.concatenate([np.arange(h * 128, (h + 1) * 128) for h in hs])
        sl = slopes16[hs]
        inv = np.exp(-2.0 * log_scale[hs]) / np.sqrt(128.0)
        sg = 1.0 / (1.0 + np.exp(-smear_factor[hs]))
        om = 1.0 - sg
        ratio = np.exp(smear_factor[hs])

        wq = w_in[:, 0 * DEXP + cols] * ln1_g[:, None]
        wk = w_in[:, 1 * DEXP + cols] * ln1_g[:, None]
        wv = w_in[:, 2 * DEXP + cols] * ln1_g[:, None]
        wp = w_in[:, 3 * DEXP + cols] * ln1_g[:, None]
        cq = ln1_b @ w_in[:, 0 * DEXP + cols]
        ck = ln1_b @ w_in[:, 1 * DEXP + cols]
        cv = ln1_b @ w_in[:, 2 * DEXP + cols]
        cp = ln1_b @ w_in[:, 3 * DEXP + cols]
        for i in range(NH):
            s = slice(i * 128, (i + 1) * 128)
            wq[:, s] *= inv[i]
            wk[:, s] *= om[i]
            cq[s] *= inv[i]
            ck[s] *= om[i]
        ccols = np.stack(
            [cq[i * 128 : (i + 1) * 128] for i in range(NH)]
            + [ck[i * 128 : (i + 1) * 128] for i in range(NH)]
            + [cp[i * 128 : (i + 1) * 128] for i in range(NH)],
            axis=1,
        ).astype(np.float32)
        ratio_t = np.tile(ratio.reshape(1, NH), (128, 1)).astype(np.float32)
        bias_cols = [sl[0] * (iota - 128.0 * d - 63.0) for d in range(3)]
        bias_cols += [sl[1] * (iota - 128.0 * dd - 447.0) for dd in range(-3, 13)]
        biasv = np.stack(bias_cols, axis=1).astype(np.float32)
        smallf = np.concatenate([ccols, ratio_t, biasv], axis=1).astype(np.float32)
        cvbc = np.tile(cv.reshape(1, DL), (128, 1))
        smallb = np.concatenate(
            [tri.astype(np.float32), iden.astype(np.float32), cvbc], axis=1
        ).astype(NP_BF16)
        g2b2 = np.concatenate(
            [np.tile(ln2_g.reshape(1, D), (128, 1)), np.tile(ln2_b.reshape(1, D), (128, 1))],
            axis=1,
        ).astype(NP_BF16)

        m = {
            "xt": np.ascontiguousarray(x[b].T).astype(NP_BF16),
            "wq": np.ascontiguousarray(wq).astype(NP_BF16),
            "wk": np.ascontiguousarray(wk).astype(NP_BF16),
            "wv": np.ascontiguousarray(wv).astype(NP_BF16),
            "wp": np.ascontiguousarray(wp).astype(NP_BF16),
            "wout": np.ascontiguousarray(w_out[cols, :]).astype(NP_BF16),
            "smallf": smallf,
            "smallb": smallb,
            "g2b2": g2b2,
        }
        in_maps.append(m)

    res = None
    last_exc = None
    for _attempt in range(3):
        try:
            res = run_bass_kernel_spmd(nc, in_maps, core_ids=list(range(8)))
            break
        except Exception as e:  # transient axon worker drops; retry
            last_exc = e
            import time as _time

            _time.sleep(2.0)
    if res is None:
        raise last_exc
    _CACHED["last_res"] = res
    out = np.empty((B, L, D), np.float32)
    for c in range(8):
        b, r = divmod(c, 4)
        o = np.asarray(res.results[c]["out"], np.float32)  # [512, 1024]
        for g in range(NG):
            out[b, 512 * g + 128 * r : 512 * g + 128 * r + 128, :] = o[
                128 * g : 128 * (g + 1), :
            ]
    return out


# revision 9
# speedup vs baseline: 1.0046x; 1.0046x over previous
"""Trainium2 Bass kernel for nn_Block_65755949302136 (dense transformer block).

Sharding: 8 cores = 2 (batch) x 4 (tensor-parallel ranks). Rank r owns heads
[r, 4+r, 8+2r, 9+2r] (slot0 = strongly-sloped ALiBi head with a 3-block
causal window, slot1 = weakly-sloped full-causal head, slots 2/3 zero-slope),
the matching w_in column slices and w_out row slice. ReduceScatter(add) over
each batch group after out_proj, LN2 on each rank's 512-row shard.

v2 design:
- LN1 gamma, per-head q/k scales and ln1_beta column corrections are folded
  into the weights on the host.
- LN1 stats are broadcast [128,512] matmuls (ones/D stationary) so the
  var/rsqrt chain runs partition-parallel; x is centered+scaled in place
  (xn = (x-mu)*rstd), removing all extended-contraction matmuls.
- x is loaded chunk-major (4 DMAs of [128, 8x512]) so stats/xn/v/qkvp
  pipeline per 512-token chunk; weights load as one DMA per kind.
- Softmax denominator rides the AV matmul: per 128-query tile the stationary
  is the exp tile and the moving operand is [v | ones] (129 cols), giving a
  token-major o plus its denominator column in one pass; the normalized,
  beta-corrected o is transposed back to feature-major on the PE and gated
  into silu(p).
- Analytic per-(q-tile,k-block) shift rides the ACT exp bias.
"""

import sys

sys.path.insert(0, "/opt/trn_rl_repo")

from collections import deque

import numpy as np

import concourse.bass as bass
import concourse.mybir as mybir
import concourse.tile as tile
from concourse.bass_utils import run_bass_kernel_spmd

F32 = mybir.dt.float32
BF16 = mybir.dt.bfloat16
NP_BF16 = mybir.dt.np(BF16)
AF = mybir.ActivationFunctionType
ALU = mybir.AluOpType

B, L, D, NHEADS, DH = 2, 2048, 1024, 16, 128
DEXP = 2048  # full d_expanded
NH = 4  # heads per core
DL = NH * DH  # 512, local d_expanded slice
KT = D // 128  # 8 k-tiles over d_model
NCH = L // 512  # 4 query chunks
NMT = L // 128  # 16 token tiles
NG = 4  # reduce-scatter groups (512 rows each)

# per-slot causal block window (slot0 = heads 0-3, min slope 0.0928 -> 3 blocks)
WB = {0: 2, 1: 16, 2: 16, 3: 16}

_CACHED = {}


def _normalize_waits(nc):
    """walrus wait-slot limits are tighter than what Tile emits for some
    instruction classes; move excess sync-waits onto same-engine NoOp
    carriers inserted immediately before the instruction."""
    for func in nc.m.functions:
        for blk in func.blocks:
            insts = blk.instructions
            i = 0
            while i < len(insts):
                inst = insts[i]
                si = inst.sync_info
                cap = 1
                if si is not None and len(si.on_wait or []) > cap:
                    waits = list(si.on_wait)
                    excess, keep = waits[:-cap], waits[-cap:]
                    for j, w in enumerate(excess):
                        d = mybir.InstNoOp(
                            name=f"{inst.name}-wsplit{j}",
                            engine=inst.engine,
                            ins=[],
                            outs=[],
                        )
                        d.sync_info = mybir.SyncInfo(on_wait=[w], on_update=[])
                        insts.insert(i, d)
                        nc.register_instruction(d, overwrite=True)
                        i += 1
                    si.on_wait = keep
                i += 1


def build(with_cc=True):
    nc = bass.Bass()

    xt_d = nc.dram_tensor("xt", [D, L], BF16, kind="ExternalInput")
    wq_d = nc.dram_tensor("wq", [D, DL], BF16, kind="ExternalInput")
    wk_d = nc.dram_tensor("wk", [D, DL], BF16, kind="ExternalInput")
    wv_d = nc.dram_tensor("wv", [D, DL], BF16, kind="ExternalInput")
    wp_d = nc.dram_tensor("wp", [D, DL], BF16, kind="ExternalInput")
    wout_d = nc.dram_tensor("wout", [DL, D], BF16, kind="ExternalInput")
    smallf_d = nc.dram_tensor("smallf", [128, 35], F32, kind="ExternalInput")
    smallb_d = nc.dram_tensor("smallb", [128, 256 + DL], BF16, kind="ExternalInput")
    g2b2_d = nc.dram_tensor("g2b2", [128, 2 * D], BF16, kind="ExternalInput")
    out_d = nc.dram_tensor("out", [NG * 128, D], BF16, kind="ExternalOutput")

    with tile.TileContext(nc, pool_alloc_mode="queue") as tc:
        cp_cm = tc.tile_pool(name="const", bufs=1)
        cp = cp_cm.__enter__()
        xbp_cm = tc.tile_pool(name="xbp", bufs=1)
        xbp = xbp_cm.__enter__()
        wp_cm = tc.tile_pool(name="wpool", bufs=1)
        wpo = wp_cm.__enter__()
        wop_cm = tc.tile_pool(name="wo", bufs=1)
        wop = wop_cm.__enter__()
        dram_cm = tc.tile_pool(name="dram", bufs=1, space="DRAM")
        dram = dram_cm.__enter__()

        # ---- big DMAs, ordered by first use ----
        xch = []
        xch3 = []
        w_t = {}
        w3 = {}

        def dma_x(ch, split=1):
            t = xbp.tile([128, KT * 512], BF16, tag="x", bufs=NCH, name=f"xch{ch}")
            csl = slice(ch * 512, (ch + 1) * 512)
            t3 = t[:, :].rearrange("p (a n) -> p a n", n=512)
            s3 = xt_d[:, csl].rearrange("(a p) n -> p a n", p=128)
            step = KT // split
            for i in range(split):
                nc.sync.dma_start(
                    t3[:, i * step : (i + 1) * step, :],
                    s3[:, i * step : (i + 1) * step, :],
                )
            xch.append(t)
            xch3.append(t3)

        def dma_w(kind, wd):
            t = wpo.tile([128, KT * DL], BF16, tag="w", bufs=4, name=f"w{kind}")
            nc.sync.dma_start(
                t[:, :].rearrange("p (a n) -> p a n", n=DL),
                wd[:, :].rearrange("(a p) n -> p a n", p=128),
            )
            w_t[kind] = t
            w3[kind] = t[:, :].rearrange("p (a n) -> p a n", n=DL)

        dma_x(0, split=4)
        dma_w("v", wv_d)
        dma_x(1, split=2)
        dma_w("q", wq_d)
        dma_w("k", wk_d)
        dma_w("p", wp_d)
        dma_x(2, split=2)
        dma_x(3, split=2)
        smallb = cp.tile([128, 256 + DL], BF16, tag="smallb")
        nc.sync.dma_start(smallb[:], smallb_d[:, :])
        tri = smallb[:, 0:128]
        iden = smallb[:, 128:256]
        cvbc = smallb[:, 256 : 256 + DL]
        smallf = cp.tile([128, 35], F32, tag="smallf")
        nc.sync.dma_start(smallf[:], smallf_d[:, :])
        ccols = smallf[:, 0:12]
        ratio = smallf[:, 12:16]
        biasv = smallf[:, 16:35]


        woutT = wop.tile([128, NH * D], BF16, tag="woutT")
        nc.sync.dma_start(
            woutT[:, :].rearrange("p (a n) -> p a n", n=D),
            wout_d[:, :].rearrange("(a p) n -> p a n", p=128),
        )
        wo3 = woutT[:, :].rearrange("p (a n) -> p a n", n=D)
        g2b2 = wop.tile([128, 2 * D], BF16, tag="g2b2")
        nc.sync.dma_start(g2b2[:], g2b2_d[:, :])
        g2bc = g2b2[:, 0:D]
        b2bc = g2b2[:, D : 2 * D]

        onesD = cp.tile([128, 128], BF16, tag="onesD")
        nc.gpsimd.memset(onesD[:], 1.0 / D)
        eps128 = cp.tile([128, 1], F32, tag="eps128")
        nc.gpsimd.memset(eps128[:], 1e-5)
        warmup_n = 15

        # ---- main pools ----
        qkp_cm = tc.tile_pool(name="qkp", bufs=1)
        qkp = qkp_cm.__enter__()
        gp_cm = tc.tile_pool(name="gp", bufs=1)
        gp = gp_cm.__enter__()
        vmp_cm = tc.tile_pool(name="vmp", bufs=1)
        vmp = vmp_cm.__enter__()
        etp_cm = tc.tile_pool(name="etp", bufs=30)
        etp = etp_cm.__enter__()
        onp_cm = tc.tile_pool(name="onp", bufs=8)
        onp = onp_cm.__enter__()
        lnp_cm = tc.tile_pool(name="ln2", bufs=1)
        lnp = lnp_cm.__enter__()
        osp_cm = tc.tile_pool(name="ostage", bufs=4)
        osp = osp_cm.__enter__()

        # psum pool for main matmuls; coexists with the stage-1 stats pool
        pmm_cm = tc.tile_pool(name="ps_mm", bufs=2, space="PSUM")
        pmm = pmm_cm.__enter__()
        # pss/po/pot are created after the stats pools close (assigned below,
        # before first use; the emitters close over these names)
        pss = po = pot = None
        pstat = xsqp = scr = srp = None

        def emit_stats(ch):
            mu_ps = pstat.tile([128, 512], F32, tag="mu", name=f"mu{ch}")
            ms_ps = pstat.tile([128, 512], F32, tag="ms", name=f"ms{ch}")
            for kt in range(KT):
                nc.tensor.matmul(
                    mu_ps[:], onesD[:], xch3[ch][:, kt, :],
                    start=(kt == 0), stop=(kt == KT - 1),
                )
            for quar in range(4):
                xsq = xsqp.tile(
                    [128, 2 * 512], BF16, tag="xsq", bufs=3, name=f"xsq{ch}_{quar}"
                )
                xsq3 = xsq[:, :].rearrange("p (a n) -> p a n", n=512)
                nc.scalar.activation(
                    xsq3[:, :, :], xch3[ch][:, 2 * quar : 2 * quar + 2, :], AF.Square
                )
                for kt in range(2):
                    nc.tensor.matmul(
                        ms_ps[:], onesD[:], xsq3[:, kt, :],
                        start=(quar == 0 and kt == 0), stop=(quar == 3 and kt == 1),
                    )
            t1 = scr.tile([128, 512], F32, tag="t1")
            nc.scalar.activation(t1[:], mu_ps[:], AF.Square)
            nc.vector.tensor_sub(t1[:], ms_ps[:], t1[:])
            nc.scalar.activation(t1[:], t1[:], AF.Sqrt, bias=eps128[:])
            rs_bc = srp.tile([128, 512], BF16, tag="rs_bc", name=f"rs{ch}")
            with nc.allow_low_precision("bf16 rstd feeds bf16 muls"):
                nc.vector.reciprocal(rs_bc[:], t1[:])
            mu_bc = srp.tile([128, 512], BF16, tag="mu_bc", name=f"mubc{ch}")
            nc.scalar.copy(mu_bc[:], mu_ps[:])
            mu_b = mu_bc[:, :].rearrange("p (a n) -> p a n", a=1).broadcast_to(
                (128, KT, 512)
            )
            rs_b = rs_bc[:, :].rearrange("p (a n) -> p a n", a=1).broadcast_to(
                (128, KT, 512)
            )
            nc.vector.tensor_sub(xch3[ch][:], xch3[ch][:], mu_b)
            nc.vector.tensor_mul(xch3[ch][:], xch3[ch][:], rs_b)

        qT = [None] * NH
        kS = [None] * NH
        geff = [None] * NH
        _kk = [None] * NH
        vm3 = [None] * NMT
        rs_in = [dram.tile([512, D], BF16, tag=f"rsin{g}", name=f"rsin{g}") for g in range(NG)]
        rs_out = [dram.tile([128, D], BF16, tag=f"rsout{g}", name=f"rsout{g}") for g in range(NG)]

        # deferral machinery
        gqueue = deque()
        carry = []  # pending per-qt normalization closures

        def flush_carry():
            while carry:
                carry.pop(0)()

        def pop1():
            gqueue.popleft()()

        def emit_v_ch(ch):
            for mi in range(4):
                m = 4 * ch + mi
                msl = slice(mi * 128, (mi + 1) * 128)
                vps = pmm.tile([128, 512], F32, tag="mm", name=f"vps{m}")
                for kt in range(KT):
                    nc.tensor.matmul(
                        vps[:], xch3[ch][:, kt, msl], w3["v"][:, kt, :],
                        start=(kt == 0), stop=(kt == KT - 1),
                    )
                t = vmp.tile([128, NH * 129], BF16, tag="vm", bufs=NMT, name=f"vm{m}")
                t3 = t[:, :].rearrange("p (a b) -> p a b", b=129)
                nc.gpsimd.memset(t3[:, :, 128:129], 1.0)
                nc.scalar.copy(t3[:, :, 0:128], vps[:])
                vm3[m] = t3

        def emit_qkvp_chunk(h, ch):
            hsl = slice(h * 128, (h + 1) * 128)
            csl = slice(ch * 512, (ch + 1) * 512)
            if ch == 0:
                qT[h] = qkp.tile([128, L], BF16, tag="qT", bufs=3, name=f"qT{h}")
                kS[h] = qkp.tile([128, L], BF16, tag="kS", bufs=3, name=f"kS{h}")
                geff[h] = gp.tile([128, L], BF16, tag="geff", bufs=NH, name=f"geff{h}")
                _kk[h] = qkp.tile([128, L], BF16, tag="kk", bufs=2, name=f"kk{h}")
            kk = _kk[h]
            qps = pmm.tile([128, 512], F32, tag="mm", name=f"qps{h}_{ch}")
            for kt in range(KT):
                nc.tensor.matmul(qps[:], w3["q"][:, kt, hsl], xch3[ch][:, kt, :],
                                 start=(kt == 0), stop=(kt == KT - 1))
            nc.vector.tensor_scalar_add(qT[h][:, csl], qps[:], ccols[:, h : h + 1])
            kps = pmm.tile([128, 512], F32, tag="mm", name=f"kps{h}_{ch}")
            for kt in range(KT):
                nc.tensor.matmul(kps[:], w3["k"][:, kt, hsl], xch3[ch][:, kt, :],
                                 start=(kt == 0), stop=(kt == KT - 1))
            nc.vector.tensor_scalar_add(kk[:, csl], kps[:], ccols[:, NH + h : NH + h + 1])
            pps = pmm.tile([128, 512], F32, tag="mm", name=f"pps{h}_{ch}")
            for kt in range(KT):
                nc.tensor.matmul(pps[:], w3["p"][:, kt, hsl], xch3[ch][:, kt, :],
                                 start=(kt == 0), stop=(kt == KT - 1))
            nc.scalar.activation(
                geff[h][:, csl], pps[:], AF.Silu,
                bias=ccols[:, 2 * NH + h : 2 * NH + h + 1],
            )

        def emit_smear(h):
            kk = _kk[h]
            nc.vector.scalar_tensor_tensor(
                kS[h][:, 1:L], kk[:, 0 : L - 1], ratio[:, h : h + 1], kk[:, 1:L],
                ALU.mult, ALU.add,
            )
            nc.vector.tensor_copy(kS[h][:, 0:1], kk[:, 0:1])

        et_info = {}

        def emit_S(h, ch):
            wb = WB[h]
            csl = slice(ch * 512, (ch + 1) * 512)
            kb_lo = max(0, 4 * ch + 1 - wb)
            kb_hi = 4 * ch + 3
            info = {}
            for kb in range(kb_lo, kb_hi + 1):
                qs0 = max(0, kb - 4 * ch)
                qs1 = min(4, kb - 4 * ch + wb)
                if qs0 >= qs1:
                    continue
                w = (qs1 - qs0) * 128
                nsl = slice(csl.start + qs0 * 128, csl.start + qs1 * 128)
                sps = pss.tile([128, w], F32, tag="sps", name=f"sps{h}_{ch}_{kb}")
                nc.tensor.matmul(
                    sps[:], kS[h][:, kb * 128 : (kb + 1) * 128], qT[h][:, nsl],
                    start=True, stop=True,
                )
                et = etp.tile([128, w], BF16, tag="et", name=f"et{h}_{ch}_{kb}")
                if h == 0:
                    for qs in range(qs0, qs1):
                        esl = slice((qs - qs0) * 128, (qs - qs0 + 1) * 128)
                        dd = (4 * ch + qs) - kb
                        nc.scalar.activation(
                            et[:, esl], sps[:, esl], AF.Exp,
                            bias=biasv[:, dd : dd + 1],
                        )
                elif h == 1:
                    dd = 4 * ch - kb + 3  # in [0, 15]
                    nc.scalar.activation(
                        et[:, :], sps[:, :], AF.Exp, bias=biasv[:, 3 + dd : 4 + dd]
                    )
                else:
                    nc.scalar.activation(et[:, :], sps[:, :], AF.Exp)
                qs_diag = kb - 4 * ch
                if qs0 <= qs_diag < qs1:
                    esl = slice((qs_diag - qs0) * 128, (qs_diag - qs0 + 1) * 128)
                    nc.gpsimd.tensor_mul(et[:, esl], et[:, esl], tri[:])
                info[kb] = (et, qs0)
            et_info[(h, ch)] = info

        def make_norm(h, qt, o):
            def norm():
                dinv = onp.tile([128, 1], F32, tag="dinv", name=f"dinv{h}_{qt}")
                nc.vector.reciprocal(dinv[:], o[:, 128:129])
                onrm = onp.tile([128, 128], BF16, tag="onrm", name=f"onrm{h}_{qt}")
                nc.vector.scalar_tensor_tensor(
                    onrm[:], o[:, 0:128], dinv[:],
                    cvbc[:, h * 128 : (h + 1) * 128], ALU.mult, ALU.add,
                )
                ot = pot.tile([128, 128], BF16, tag="ot", name=f"ot{h}_{qt}")
                nc.tensor.transpose(ot[:], onrm[:], iden[:])
                msl = slice(qt * 128, (qt + 1) * 128)
                nc.vector.tensor_mul(geff[h][:, msl], ot[:], geff[h][:, msl])

            return norm

        def make_AV(h, ch):
            def av():
                flush_carry()
                wb = WB[h]
                info = et_info.pop((h, ch))
                for qs in range(4):
                    qt = 4 * ch + qs
                    kbs = list(range(max(0, qt - wb + 1), qt + 1))
                    o = po.tile([128, 129], F32, tag="o", name=f"o{h}_{qt}")
                    for kb in kbs:
                        et, qs0 = info[kb]
                        esl = slice((qs - qs0) * 128, (qs - qs0 + 1) * 128)
                        nc.tensor.matmul(
                            o[:], et[:, esl], vm3[kb][:, h, :],
                            start=(kb == kbs[0]), stop=(kb == kbs[-1]),
                        )
                    carry.append(make_norm(h, qt, o))
                    if len(carry) >= 3:
                        carry.pop(0)()

            return av

        def emit_op(g):
            flush_carry()
            for mi in range(4):
                m = 4 * g + mi
                msl = slice(m * 128, (m + 1) * 128)
                for n2 in range(2):
                    nsl2 = slice(n2 * 512, (n2 + 1) * 512)
                    op2 = pmm.tile([128, 512], F32, tag="mm", name=f"op2_{m}_{n2}")
                    for hh in range(NH):
                        nc.tensor.matmul(
                            op2[:], geff[hh][:, msl], wo3[:, hh, nsl2],
                            start=(hh == 0), stop=(hh == NH - 1),
                        )
                    osb = osp.tile([128, 512], BF16, tag="osb")
                    if g == NG - 1:
                        nc.scalar.copy(osb[:], op2[:])
                    else:
                        nc.vector.tensor_copy(osb[:], op2[:])
                    nc.sync.dma_start(rs_in[g][mi * 128 : (mi + 1) * 128, nsl2], osb[:])
            if with_cc:
                nc.gpsimd.collective_compute(
                    "ReduceScatter", ALU.add,
                    replica_groups=[[0, 1, 2, 3], [4, 5, 6, 7]],
                    ins=[rs_in[g][:, :].opt()],
                    outs=[rs_out[g][:, :].opt()],
                )
            else:
                nc.sync.dma_start(rs_out[g][:, :], rs_in[g][0:128, :])
            yt = lnp.tile([128, D], BF16, tag="yt")
            bs = lnp.tile([128, 12], F32, tag="bs")
            for half in range(2):
                hsl2 = slice(half * 512, (half + 1) * 512)
                nc.sync.dma_start(yt[:, hsl2], rs_out[g][:, hsl2])
                nc.vector.bn_stats(bs[:, 6 * half : 6 * half + 6], yt[:, hsl2])
            ag = lnp.tile([128, 2], F32, tag="ag")
            nc.vector.bn_aggr(ag[:], bs[:])
            sd2 = lnp.tile([128, 1], F32, tag="sd2")
            nc.scalar.activation(sd2[:], ag[:, 1:2], AF.Sqrt, bias=eps128[:])
            rstd2 = lnp.tile([128, 1], F32, tag="rstd2")
            nc.vector.reciprocal(rstd2[:], sd2[:])
            nmu = lnp.tile([128, 1], F32, tag="nmu")
            nc.vector.tensor_scalar_mul(nmu[:], ag[:, 0:1], -1.0)
            t2 = lnp.tile([128, D], BF16, tag="t2")
            for half in range(2):
                hsl2 = slice(half * 512, (half + 1) * 512)
                nc.vector.tensor_scalar(
                    t2[:, hsl2], yt[:, hsl2], nmu[:], rstd2[:], ALU.add, ALU.mult
                )
                nc.vector.tensor_mul(t2[:, hsl2], t2[:, hsl2], g2bc[:, hsl2])
                nc.vector.tensor_add(t2[:, hsl2], t2[:, hsl2], b2bc[:, hsl2])
            nc.sync.dma_start(out_d[g * 128 : (g + 1) * 128, :], t2[:])

        # ---- emission schedule ----
        st_cms = [
            tc.tile_pool(name="ps_stat", bufs=2, space="PSUM"),
            tc.tile_pool(name="xsqp", bufs=1),
            tc.tile_pool(name="scr", bufs=1),
            tc.tile_pool(name="srow", bufs=2),
        ]
        pstat, xsqp, scr, srp = [cm.__enter__() for cm in st_cms]
        for wi in range(warmup_n):
            wps = pmm.tile([128, 128], F32, tag="mm", name=f"warm{wi}")
            nc.tensor.matmul(wps[:], onesD[:], onesD[:], start=True, stop=True)
        emit_stats(0)
        emit_stats(1)
        emit_v_ch(0)
        emit_qkvp_chunk(0, 0)
        emit_qkvp_chunk(1, 0)
        emit_stats(2)
        emit_v_ch(1)
        emit_qkvp_chunk(0, 1)
        emit_qkvp_chunk(1, 1)
        emit_stats(3)
        emit_v_ch(2)
        emit_qkvp_chunk(0, 2)
        emit_qkvp_chunk(1, 2)
        emit_v_ch(3)
        emit_qkvp_chunk(0, 3)
        emit_qkvp_chunk(1, 3)
        for cm in reversed(st_cms):
            cm.__exit__(None, None, None)
        emit_smear(0)
        emit_smear(1)
        pss_cm = tc.tile_pool(name="ps_s", bufs=3, space="PSUM")
        pss = pss_cm.__enter__()
        po_cm = tc.tile_pool(name="ps_o", bufs=2, space="PSUM")
        po = po_cm.__enter__()
        pot_cm = tc.tile_pool(name="ps_ot", bufs=1, space="PSUM")
        pot = pot_cm.__enter__()
        # E1: attn0 interleaved with qkvp2
        for ch in range(NCH):
            emit_S(0, ch)
            gqueue.append(make_AV(0, ch))
            emit_qkvp_chunk(2, ch)
            if len(gqueue) > 2:
                pop1()
        emit_smear(2)
        # E2: attn1 interleaved with qkvp3 (defer depth 2)
        for ch in range(NCH):
            emit_S(1, ch)
            gqueue.append(make_AV(1, ch))
            emit_qkvp_chunk(3, ch)
            if len(gqueue) > 2:
                pop1()
        emit_smear(3)
        # E3: attn2 + attn3 + out_proj, pipelined
        for ch in range(NCH):
            emit_S(2, ch)
            gqueue.append(make_AV(2, ch))
            pop1()
            if ch >= 2:
                emit_op(ch - 2)
            emit_S(3, ch)
            gqueue.append(make_AV(3, ch))
            pop1()
        emit_op(NG - 2)
        pop1()
        pop1()
        assert not gqueue
        emit_op(NG - 1)
        flush_carry()

        pot_cm.__exit__(None, None, None)
        po_cm.__exit__(None, None, None)
        pss_cm.__exit__(None, None, None)
        pmm_cm.__exit__(None, None, None)
        osp_cm.__exit__(None, None, None)
        lnp_cm.__exit__(None, None, None)
        onp_cm.__exit__(None, None, None)
        etp_cm.__exit__(None, None, None)
        vmp_cm.__exit__(None, None, None)
        gp_cm.__exit__(None, None, None)
        qkp_cm.__exit__(None, None, None)
        dram_cm.__exit__(None, None, None)
        wop_cm.__exit__(None, None, None)
        wp_cm.__exit__(None, None, None)
        xbp_cm.__exit__(None, None, None)
        cp_cm.__exit__(None, None, None)

    _normalize_waits(nc)
    return nc


def _slopes16():
    half = NHEADS // 2
    return np.concatenate(
        [2.0 ** np.linspace(0.0, -8.0, half), np.zeros(NHEADS - half)]
    ).astype(np.float32)


def kernel(x, ln1_g, ln1_b, ln2_g, ln2_b, w_in, w_out, smear_factor, log_scale):
    x = np.asarray(x, np.float32)
    w_in = np.asarray(w_in, np.float32)
    w_out = np.asarray(w_out, np.float32)
    ln1_g = np.asarray(ln1_g, np.float32)
    ln1_b = np.asarray(ln1_b, np.float32)
    ln2_g = np.asarray(ln2_g, np.float32)
    ln2_b = np.asarray(ln2_b, np.float32)
    smear_factor = np.asarray(smear_factor, np.float32)
    log_scale = np.asarray(log_scale, np.float32)

    if "nc" not in _CACHED:
        _CACHED["nc"] = build()
    nc = _CACHED["nc"]

    slopes16 = _slopes16()
    jj = np.arange(128)
    tri = (jj[:, None] <= jj[None, :]).astype(NP_BF16)  # keep j <= i
    iden = np.eye(128, dtype=NP_BF16)
    iota = np.arange(128, dtype=np.float32)

    in_maps = []
    for c in range(8):
        b, r = divmod(c, 4)
        hs = [r, 4 + r, 8 + 2 * r, 9 + 2 * r]
        cols = np.concatenate([np.arange(h * 128, (h + 1) * 128) for h in hs])
        sl = slopes16[hs]
        inv = np.exp(-2.0 * log_scale[hs]) / np.sqrt(128.0)
        sg = 1.0 / (1.0 + np.exp(-smear_factor[hs]))
        om = 1.0 - sg
        ratio = np.exp(smear_factor[hs])

        wq = w_in[:, 0 * DEXP + cols] * ln1_g[:, None]
        wk = w_in[:, 1 * DEXP + cols] * ln1_g[:, None]
        wv = w_in[:, 2 * DEXP + cols] * ln1_g[:, None]
        wp = w_in[:, 3 * DEXP + cols] * ln1_g[:, None]
        cq = ln1_b @ w_in[:, 0 * DEXP + cols]
        ck = ln1_b @ w_in[:, 1 * DEXP + cols]
        cv = ln1_b @ w_in[:, 2 * DEXP + cols]
        cp = ln1_b @ w_in[:, 3 * DEXP + cols]
        for i in range(NH):
            s = slice(i * 128, (i + 1) * 128)
            wq[:, s] *= inv[i]
            wk[:, s] *= om[i]
            cq[s] *= inv[i]
            ck[s] *= om[i]
        ccols = np.stack(
            [cq[i * 128 : (i + 1) * 128] for i in range(NH)]
            + [ck[i * 128 : (i + 1) * 128] for i in range(NH)]
            + [cp[i * 128 : (i + 1) * 128] for i in range(NH)],
            axis=1,
        ).astype(np.float32)
        ratio_t = np.tile(ratio.reshape(1, NH), (128, 1)).astype(np.float32)
        bias_cols = [sl[0] * (iota - 128.0 * d - 63.0) for d in range(3)]
        bias_cols += [sl[1] * (iota - 128.0 * dd - 447.0) for dd in range(-3, 13)]
        biasv = np.stack(bias_cols, axis=1).astype(np.float32)
        smallf = np.concatenate([ccols, ratio_t, biasv], axis=1).astype(np.float32)
        cvbc = np.tile(cv.reshape(1, DL), (128, 1))
        smallb = np.concatenate(
            [tri.astype(np.float32), iden.astype(np.float32), cvbc], axis=1
        ).astype(NP_BF16)
        g2b2 = np.concatenate(
            [np.tile(ln2_g.reshape(1, D), (128, 1)), np.tile(ln2_b.reshape(1, D), (128, 1))],
            axis=1,
        ).astype(NP_BF16)

        m = {
            "xt": np.ascontiguousarray(x[b].T).astype(NP_BF16),
            "wq": np.ascontiguousarray(wq).astype(NP_BF16),
            "wk": np.ascontiguousarray(wk).astype(NP_BF16),
            "wv": np.ascontiguousarray(wv).astype(NP_BF16),
            "wp": np.ascontiguousarray(wp).astype(NP_BF16),
            "wout": np.ascontiguousarray(w_out[cols, :]).astype(NP_BF16),
            "smallf": smallf,
            "smallb": smallb,
            "g2b2": g2b2,
        }
        in_maps.append(m)

    res = None
    last_exc = None
    for _attempt in range(3):
        try:
            res = run_bass_kernel_spmd(nc, in_maps, core_ids=list(range(8)))
            break
        except Exception as e:  # transient axon worker drops; retry
            last_exc = e
            import time as _time

            _time.sleep(2.0)
    if res is None:
        raise last_exc
    _CACHED["last_res"] = res
    out = np.empty((B, L, D), np.float32)
    for c in range(8):
        b, r = divmod(c, 4)
        o = np.asarray(res.results[c]["out"], np.float32)  # [512, 1024]
        for g in range(NG):
            out[b, 512 * g + 128 * r : 512 * g + 128 * r + 128, :] = o[
                128 * g : 128 * (g + 1), :
            ]
    return out


# revision 10
# speedup vs baseline: 1.0054x; 1.0007x over previous
"""Trainium2 Bass kernel for nn_Block_65755949302136 (dense transformer block).

Sharding: 8 cores = 2 (batch) x 4 (tensor-parallel ranks). Rank r owns heads
[r, 4+r, 8+2r, 9+2r] (slot0 = strongly-sloped ALiBi head with a 3-block
causal window, slot1 = weakly-sloped full-causal head, slots 2/3 zero-slope),
the matching w_in column slices and w_out row slice. ReduceScatter(add) over
each batch group after out_proj, LN2 on each rank's 512-row shard.

v2 design:
- LN1 gamma, per-head q/k scales and ln1_beta column corrections are folded
  into the weights on the host.
- LN1 stats are broadcast [128,512] matmuls (ones/D stationary) so the
  var/rsqrt chain runs partition-parallel; x is centered+scaled in place
  (xn = (x-mu)*rstd), removing all extended-contraction matmuls.
- x is loaded chunk-major (4 DMAs of [128, 8x512]) so stats/xn/v/qkvp
  pipeline per 512-token chunk; weights load as one DMA per kind.
- Softmax denominator rides the AV matmul: per 128-query tile the stationary
  is the exp tile and the moving operand is [v | ones] (129 cols), giving a
  token-major o plus its denominator column in one pass; the normalized,
  beta-corrected o is transposed back to feature-major on the PE and gated
  into silu(p).
- Analytic per-(q-tile,k-block) shift rides the ACT exp bias.
"""

import sys

sys.path.insert(0, "/opt/trn_rl_repo")

from collections import deque

import numpy as np

import concourse.bass as bass
import concourse.mybir as mybir
import concourse.tile as tile
from concourse.bass_utils import run_bass_kernel_spmd

F32 = mybir.dt.float32
BF16 = mybir.dt.bfloat16
NP_BF16 = mybir.dt.np(BF16)
AF = mybir.ActivationFunctionType
ALU = mybir.AluOpType

B, L, D, NHEADS, DH = 2, 2048, 1024, 16, 128
DEXP = 2048  # full d_expanded
NH = 4  # heads per core
DL = NH * DH  # 512, local d_expanded slice
KT = D // 128  # 8 k-tiles over d_model
NCH = L // 512  # 4 query chunks
NMT = L // 128  # 16 token tiles
NG = 4  # reduce-scatter groups (512 rows each)

# per-slot causal block window (slot0 = heads 0-3, min slope 0.0928 -> 3 blocks)
WB = {0: 2, 1: 16, 2: 16, 3: 16}

_CACHED = {}


def _normalize_waits(nc):
    """walrus wait-slot limits are tighter than what Tile emits for some
    instruction classes; move excess sync-waits onto same-engine NoOp
    carriers inserted immediately before the instruction."""
    for func in nc.m.functions:
        for blk in func.blocks:
            insts = blk.instructions
            i = 0
            while i < len(insts):
                inst = insts[i]
                si = inst.sync_info
                cap = 1
                if si is not None and len(si.on_wait or []) > cap:
                    waits = list(si.on_wait)
                    excess, keep = waits[:-cap], waits[-cap:]
                    for j, w in enumerate(excess):
                        d = mybir.InstNoOp(
                            name=f"{inst.name}-wsplit{j}",
                            engine=inst.engine,
                            ins=[],
                            outs=[],
                        )
                        d.sync_info = mybir.SyncInfo(on_wait=[w], on_update=[])
                        insts.insert(i, d)
                        nc.register_instruction(d, overwrite=True)
                        i += 1
                    si.on_wait = keep
                i += 1


def build(with_cc=True):
    nc = bass.Bass()

    xt_d = nc.dram_tensor("xt", [D, L], BF16, kind="ExternalInput")
    wq_d = nc.dram_tensor("wq", [D, DL], BF16, kind="ExternalInput")
    wk_d = nc.dram_tensor("wk", [D, DL], BF16, kind="ExternalInput")
    wv_d = nc.dram_tensor("wv", [D, DL], BF16, kind="ExternalInput")
    wp_d = nc.dram_tensor("wp", [D, DL], BF16, kind="ExternalInput")
    wout_d = nc.dram_tensor("wout", [DL, D], BF16, kind="ExternalInput")
    smallf_d = nc.dram_tensor("smallf", [128, 35], F32, kind="ExternalInput")
    smallb_d = nc.dram_tensor("smallb", [128, 256 + DL], BF16, kind="ExternalInput")
    g2b2_d = nc.dram_tensor("g2b2", [128, 2 * D], BF16, kind="ExternalInput")
    out_d = nc.dram_tensor("out", [NG * 128, D], BF16, kind="ExternalOutput")

    with tile.TileContext(nc, pool_alloc_mode="queue") as tc:
        cp_cm = tc.tile_pool(name="const", bufs=1)
        cp = cp_cm.__enter__()
        xbp_cm = tc.tile_pool(name="xbp", bufs=1)
        xbp = xbp_cm.__enter__()
        wp_cm = tc.tile_pool(name="wpool", bufs=1)
        wpo = wp_cm.__enter__()
        wop_cm = tc.tile_pool(name="wo", bufs=1)
        wop = wop_cm.__enter__()
        dram_cm = tc.tile_pool(name="dram", bufs=1, space="DRAM")
        dram = dram_cm.__enter__()

        # ---- big DMAs, ordered by first use ----
        xch = []
        xch3 = []
        w_t = {}
        w3 = {}

        def dma_x(ch, split=1):
            t = xbp.tile([128, KT * 512], BF16, tag="x", bufs=NCH, name=f"xch{ch}")
            csl = slice(ch * 512, (ch + 1) * 512)
            t3 = t[:, :].rearrange("p (a n) -> p a n", n=512)
            s3 = xt_d[:, csl].rearrange("(a p) n -> p a n", p=128)
            step = KT // split
            for i in range(split):
                nc.sync.dma_start(
                    t3[:, i * step : (i + 1) * step, :],
                    s3[:, i * step : (i + 1) * step, :],
                )
            xch.append(t)
            xch3.append(t3)

        def dma_w(kind, wd):
            t = wpo.tile([128, KT * DL], BF16, tag="w", bufs=4, name=f"w{kind}")
            nc.sync.dma_start(
                t[:, :].rearrange("p (a n) -> p a n", n=DL),
                wd[:, :].rearrange("(a p) n -> p a n", p=128),
            )
            w_t[kind] = t
            w3[kind] = t[:, :].rearrange("p (a n) -> p a n", n=DL)

        dma_x(0, split=4)
        dma_w("v", wv_d)
        dma_x(1, split=4)
        dma_w("q", wq_d)
        dma_w("k", wk_d)
        dma_w("p", wp_d)
        dma_x(2, split=2)
        dma_x(3, split=2)
        smallb = cp.tile([128, 256 + DL], BF16, tag="smallb")
        nc.sync.dma_start(smallb[:], smallb_d[:, :])
        tri = smallb[:, 0:128]
        iden = smallb[:, 128:256]
        cvbc = smallb[:, 256 : 256 + DL]
        smallf = cp.tile([128, 35], F32, tag="smallf")
        nc.sync.dma_start(smallf[:], smallf_d[:, :])
        ccols = smallf[:, 0:12]
        ratio = smallf[:, 12:16]
        biasv = smallf[:, 16:35]


        woutT = wop.tile([128, NH * D], BF16, tag="woutT")
        nc.sync.dma_start(
            woutT[:, :].rearrange("p (a n) -> p a n", n=D),
            wout_d[:, :].rearrange("(a p) n -> p a n", p=128),
        )
        wo3 = woutT[:, :].rearrange("p (a n) -> p a n", n=D)
        g2b2 = wop.tile([128, 2 * D], BF16, tag="g2b2")
        nc.sync.dma_start(g2b2[:], g2b2_d[:, :])
        g2bc = g2b2[:, 0:D]
        b2bc = g2b2[:, D : 2 * D]

        onesD = cp.tile([128, 128], BF16, tag="onesD")
        nc.gpsimd.memset(onesD[:], 1.0 / D)
        eps128 = cp.tile([128, 1], F32, tag="eps128")
        nc.gpsimd.memset(eps128[:], 1e-5)
        warmup_n = 15

        # ---- main pools ----
        qkp_cm = tc.tile_pool(name="qkp", bufs=1)
        qkp = qkp_cm.__enter__()
        gp_cm = tc.tile_pool(name="gp", bufs=1)
        gp = gp_cm.__enter__()
        vmp_cm = tc.tile_pool(name="vmp", bufs=1)
        vmp = vmp_cm.__enter__()
        etp_cm = tc.tile_pool(name="etp", bufs=30)
        etp = etp_cm.__enter__()
        onp_cm = tc.tile_pool(name="onp", bufs=8)
        onp = onp_cm.__enter__()
        lnp_cm = tc.tile_pool(name="ln2", bufs=1)
        lnp = lnp_cm.__enter__()
        osp_cm = tc.tile_pool(name="ostage", bufs=4)
        osp = osp_cm.__enter__()

        # psum pool for main matmuls; coexists with the stage-1 stats pool
        pmm_cm = tc.tile_pool(name="ps_mm", bufs=2, space="PSUM")
        pmm = pmm_cm.__enter__()
        # pss/po/pot are created after the stats pools close (assigned below,
        # before first use; the emitters close over these names)
        pss = po = pot = None
        pstat = xsqp = scr = srp = None

        def emit_stats(ch):
            mu_ps = pstat.tile([128, 512], F32, tag="mu", name=f"mu{ch}")
            ms_ps = pstat.tile([128, 512], F32, tag="ms", name=f"ms{ch}")
            for kt in range(KT):
                nc.tensor.matmul(
                    mu_ps[:], onesD[:], xch3[ch][:, kt, :],
                    start=(kt == 0), stop=(kt == KT - 1),
                )
            for quar in range(4):
                xsq = xsqp.tile(
                    [128, 2 * 512], BF16, tag="xsq", bufs=3, name=f"xsq{ch}_{quar}"
                )
                xsq3 = xsq[:, :].rearrange("p (a n) -> p a n", n=512)
                nc.scalar.activation(
                    xsq3[:, :, :], xch3[ch][:, 2 * quar : 2 * quar + 2, :], AF.Square
                )
                for kt in range(2):
                    nc.tensor.matmul(
                        ms_ps[:], onesD[:], xsq3[:, kt, :],
                        start=(quar == 0 and kt == 0), stop=(quar == 3 and kt == 1),
                    )
            t1 = scr.tile([128, 512], F32, tag="t1")
            nc.scalar.activation(t1[:], mu_ps[:], AF.Square)
            nc.vector.tensor_sub(t1[:], ms_ps[:], t1[:])
            nc.scalar.activation(t1[:], t1[:], AF.Sqrt, bias=eps128[:])
            rs_bc = srp.tile([128, 512], BF16, tag="rs_bc", name=f"rs{ch}")
            with nc.allow_low_precision("bf16 rstd feeds bf16 muls"):
                nc.vector.reciprocal(rs_bc[:], t1[:])
            mu_bc = srp.tile([128, 512], BF16, tag="mu_bc", name=f"mubc{ch}")
            nc.scalar.copy(mu_bc[:], mu_ps[:])
            mu_b = mu_bc[:, :].rearrange("p (a n) -> p a n", a=1).broadcast_to(
                (128, KT, 512)
            )
            rs_b = rs_bc[:, :].rearrange("p (a n) -> p a n", a=1).broadcast_to(
                (128, KT, 512)
            )
            nc.vector.tensor_sub(xch3[ch][:], xch3[ch][:], mu_b)
            nc.vector.tensor_mul(xch3[ch][:], xch3[ch][:], rs_b)

        qT = [None] * NH
        kS = [None] * NH
        geff = [None] * NH
        _kk = [None] * NH
        vm3 = [None] * NMT
        rs_in = [dram.tile([512, D], BF16, tag=f"rsin{g}", name=f"rsin{g}") for g in range(NG)]
        rs_out = [dram.tile([128, D], BF16, tag=f"rsout{g}", name=f"rsout{g}") for g in range(NG)]

        # deferral machinery
        gqueue = deque()
        carry = []  # pending per-qt normalization closures

        def flush_carry():
            while carry:
                carry.pop(0)()

        def pop1():
            gqueue.popleft()()

        def emit_v_ch(ch):
            for mi in range(4):
                m = 4 * ch + mi
                msl = slice(mi * 128, (mi + 1) * 128)
                vps = pmm.tile([128, 512], F32, tag="mm", name=f"vps{m}")
                for kt in range(KT):
                    nc.tensor.matmul(
                        vps[:], xch3[ch][:, kt, msl], w3["v"][:, kt, :],
                        start=(kt == 0), stop=(kt == KT - 1),
                    )
                t = vmp.tile([128, NH * 129], BF16, tag="vm", bufs=NMT, name=f"vm{m}")
                t3 = t[:, :].rearrange("p (a b) -> p a b", b=129)
                nc.gpsimd.memset(t3[:, :, 128:129], 1.0)
                nc.scalar.copy(t3[:, :, 0:128], vps[:])
                vm3[m] = t3

        def emit_qkvp_chunk(h, ch):
            hsl = slice(h * 128, (h + 1) * 128)
            csl = slice(ch * 512, (ch + 1) * 512)
            if ch == 0:
                qT[h] = qkp.tile([128, L], BF16, tag="qT", bufs=3, name=f"qT{h}")
                kS[h] = qkp.tile([128, L], BF16, tag="kS", bufs=3, name=f"kS{h}")
                geff[h] = gp.tile([128, L], BF16, tag="geff", bufs=NH, name=f"geff{h}")
                _kk[h] = qkp.tile([128, L], BF16, tag="kk", bufs=2, name=f"kk{h}")
            kk = _kk[h]
            qps = pmm.tile([128, 512], F32, tag="mm", name=f"qps{h}_{ch}")
            for kt in range(KT):
                nc.tensor.matmul(qps[:], w3["q"][:, kt, hsl], xch3[ch][:, kt, :],
                                 start=(kt == 0), stop=(kt == KT - 1))
            nc.vector.tensor_scalar_add(qT[h][:, csl], qps[:], ccols[:, h : h + 1])
            kps = pmm.tile([128, 512], F32, tag="mm", name=f"kps{h}_{ch}")
            for kt in range(KT):
                nc.tensor.matmul(kps[:], w3["k"][:, kt, hsl], xch3[ch][:, kt, :],
                                 start=(kt == 0), stop=(kt == KT - 1))
            nc.vector.tensor_scalar_add(kk[:, csl], kps[:], ccols[:, NH + h : NH + h + 1])
            pps = pmm.tile([128, 512], F32, tag="mm", name=f"pps{h}_{ch}")
            for kt in range(KT):
                nc.tensor.matmul(pps[:], w3["p"][:, kt, hsl], xch3[ch][:, kt, :],
                                 start=(kt == 0), stop=(kt == KT - 1))
            nc.scalar.activation(
                geff[h][:, csl], pps[:], AF.Silu,
                bias=ccols[:, 2 * NH + h : 2 * NH + h + 1],
            )

        def emit_smear(h):
            kk = _kk[h]
            nc.vector.scalar_tensor_tensor(
                kS[h][:, 1:L], kk[:, 0 : L - 1], ratio[:, h : h + 1], kk[:, 1:L],
                ALU.mult, ALU.add,
            )
            nc.vector.tensor_copy(kS[h][:, 0:1], kk[:, 0:1])

        et_info = {}

        def emit_S(h, ch):
            wb = WB[h]
            csl = slice(ch * 512, (ch + 1) * 512)
            kb_lo = max(0, 4 * ch + 1 - wb)
            kb_hi = 4 * ch + 3
            info = {}
            for kb in range(kb_lo, kb_hi + 1):
                qs0 = max(0, kb - 4 * ch)
                qs1 = min(4, kb - 4 * ch + wb)
                if qs0 >= qs1:
                    continue
                w = (qs1 - qs0) * 128
                nsl = slice(csl.start + qs0 * 128, csl.start + qs1 * 128)
                sps = pss.tile([128, w], F32, tag="sps", name=f"sps{h}_{ch}_{kb}")
                nc.tensor.matmul(
                    sps[:], kS[h][:, kb * 128 : (kb + 1) * 128], qT[h][:, nsl],
                    start=True, stop=True,
                )
                et = etp.tile([128, w], BF16, tag="et", name=f"et{h}_{ch}_{kb}")
                if h == 0:
                    for qs in range(qs0, qs1):
                        esl = slice((qs - qs0) * 128, (qs - qs0 + 1) * 128)
                        dd = (4 * ch + qs) - kb
                        nc.scalar.activation(
                            et[:, esl], sps[:, esl], AF.Exp,
                            bias=biasv[:, dd : dd + 1],
                        )
                elif h == 1:
                    dd = 4 * ch - kb + 3  # in [0, 15]
                    nc.scalar.activation(
                        et[:, :], sps[:, :], AF.Exp, bias=biasv[:, 3 + dd : 4 + dd]
                    )
                else:
                    nc.scalar.activation(et[:, :], sps[:, :], AF.Exp)
                qs_diag = kb - 4 * ch
                if qs0 <= qs_diag < qs1:
                    esl = slice((qs_diag - qs0) * 128, (qs_diag - qs0 + 1) * 128)
                    nc.gpsimd.tensor_mul(et[:, esl], et[:, esl], tri[:])
                info[kb] = (et, qs0)
            et_info[(h, ch)] = info

        def make_norm(h, qt, o):
            def norm():
                dinv = onp.tile([128, 1], F32, tag="dinv", name=f"dinv{h}_{qt}")
                nc.vector.reciprocal(dinv[:], o[:, 128:129])
                onrm = onp.tile([128, 128], BF16, tag="onrm", name=f"onrm{h}_{qt}")
                nc.vector.scalar_tensor_tensor(
                    onrm[:], o[:, 0:128], dinv[:],
                    cvbc[:, h * 128 : (h + 1) * 128], ALU.mult, ALU.add,
                )
                ot = pot.tile([128, 128], BF16, tag="ot", name=f"ot{h}_{qt}")
                nc.tensor.transpose(ot[:], onrm[:], iden[:])
                msl = slice(qt * 128, (qt + 1) * 128)
                nc.vector.tensor_mul(geff[h][:, msl], ot[:], geff[h][:, msl])

            return norm

        def make_AV(h, ch):
            def av():
                flush_carry()
                wb = WB[h]
                info = et_info.pop((h, ch))
                for qs in range(4):
                    qt = 4 * ch + qs
                    kbs = list(range(max(0, qt - wb + 1), qt + 1))
                    o = po.tile([128, 129], F32, tag="o", name=f"o{h}_{qt}")
                    for kb in kbs:
                        et, qs0 = info[kb]
                        esl = slice((qs - qs0) * 128, (qs - qs0 + 1) * 128)
                        nc.tensor.matmul(
                            o[:], et[:, esl], vm3[kb][:, h, :],
                            start=(kb == kbs[0]), stop=(kb == kbs[-1]),
                        )
                    carry.append(make_norm(h, qt, o))
                    if len(carry) >= 3:
                        carry.pop(0)()

            return av

        def emit_op(g):
            flush_carry()
            for mi in range(4):
                m = 4 * g + mi
                msl = slice(m * 128, (m + 1) * 128)
                for n2 in range(2):
                    nsl2 = slice(n2 * 512, (n2 + 1) * 512)
                    op2 = pmm.tile([128, 512], F32, tag="mm", name=f"op2_{m}_{n2}")
                    for hh in range(NH):
                        nc.tensor.matmul(
                            op2[:], geff[hh][:, msl], wo3[:, hh, nsl2],
                            start=(hh == 0), stop=(hh == NH - 1),
                        )
                    osb = osp.tile([128, 512], BF16, tag="osb")
                    if g == NG - 1:
                        nc.scalar.copy(osb[:], op2[:])
                    else:
                        nc.vector.tensor_copy(osb[:], op2[:])
                    nc.sync.dma_start(rs_in[g][mi * 128 : (mi + 1) * 128, nsl2], osb[:])
            if with_cc:
                nc.gpsimd.collective_compute(
                    "ReduceScatter", ALU.add,
                    replica_groups=[[0, 1, 2, 3], [4, 5, 6, 7]],
                    ins=[rs_in[g][:, :].opt()],
                    outs=[rs_out[g][:, :].opt()],
                )
            else:
                nc.sync.dma_start(rs_out[g][:, :], rs_in[g][0:128, :])
            yt = lnp.tile([128, D], BF16, tag="yt")
            bs = lnp.tile([128, 12], F32, tag="bs")
            for half in range(2):
                hsl2 = slice(half * 512, (half + 1) * 512)
                nc.sync.dma_start(yt[:, hsl2], rs_out[g][:, hsl2])
                nc.vector.bn_stats(bs[:, 6 * half : 6 * half + 6], yt[:, hsl2])
            ag = lnp.tile([128, 2], F32, tag="ag")
            nc.vector.bn_aggr(ag[:], bs[:])
            sd2 = lnp.tile([128, 1], F32, tag="sd2")
            nc.scalar.activation(sd2[:], ag[:, 1:2], AF.Sqrt, bias=eps128[:])
            rstd2 = lnp.tile([128, 1], F32, tag="rstd2")
            nc.vector.reciprocal(rstd2[:], sd2[:])
            nmu = lnp.tile([128, 1], F32, tag="nmu")
            nc.vector.tensor_scalar_mul(nmu[:], ag[:, 0:1], -1.0)
            t2 = lnp.tile([128, D], BF16, tag="t2")
            for half in range(2):
                hsl2 = slice(half * 512, (half + 1) * 512)
                nc.vector.tensor_scalar(
                    t2[:, hsl2], yt[:, hsl2], nmu[:], rstd2[:], ALU.add, ALU.mult
                )
                nc.vector.tensor_mul(t2[:, hsl2], t2[:, hsl2], g2bc[:, hsl2])
                nc.vector.tensor_add(t2[:, hsl2], t2[:, hsl2], b2bc[:, hsl2])
            nc.sync.dma_start(out_d[g * 128 : (g + 1) * 128, :], t2[:])

        # ---- emission schedule ----
        st_cms = [
            tc.tile_pool(name="ps_stat", bufs=2, space="PSUM"),
            tc.tile_pool(name="xsqp", bufs=1),
            tc.tile_pool(name="scr", bufs=1),
            tc.tile_pool(name="srow", bufs=2),
        ]
        pstat, xsqp, scr, srp = [cm.__enter__() for cm in st_cms]
        for wi in range(warmup_n):
            wps = pmm.tile([128, 128], F32, tag="mm", name=f"warm{wi}")
            nc.tensor.matmul(wps[:], onesD[:], onesD[:], start=True, stop=True)
        emit_stats(0)
        emit_stats(1)
        emit_v_ch(0)
        emit_qkvp_chunk(0, 0)
        emit_qkvp_chunk(1, 0)
        emit_stats(2)
        emit_v_ch(1)
        emit_qkvp_chunk(0, 1)
        emit_qkvp_chunk(1, 1)
        emit_stats(3)
        emit_v_ch(2)
        emit_qkvp_chunk(0, 2)
        emit_qkvp_chunk(1, 2)
        emit_v_ch(3)
        emit_qkvp_chunk(0, 3)
        emit_qkvp_chunk(1, 3)
        for cm in reversed(st_cms):
            cm.__exit__(None, None, None)
        emit_smear(0)
        emit_smear(1)
        pss_cm = tc.tile_pool(name="ps_s", bufs=3, space="PSUM")
        pss = pss_cm.__enter__()
        po_cm = tc.tile_pool(name="ps_o", bufs=2, space="PSUM")
        po = po_cm.__enter__()
        pot_cm = tc.tile_pool(name="ps_ot", bufs=1, space="PSUM")
        pot = pot_cm.__enter__()
        # E1: attn0 interleaved with qkvp2
        for ch in range(NCH):
            emit_S(0, ch)
            gqueue.append(make_AV(0, ch))
            emit_qkvp_chunk(2, ch)
            if len(gqueue) > 2:
                pop1()
        emit_smear(2)
        # E2: attn1 interleaved with qkvp3 (defer depth 2)
        for ch in range(NCH):
            emit_S(1, ch)
            gqueue.append(make_AV(1, ch))
            emit_qkvp_chunk(3, ch)
            if len(gqueue) > 2:
                pop1()
        emit_smear(3)
        # E3: attn2 + attn3 + out_proj, pipelined
        for ch in range(NCH):
            emit_S(2, ch)
            gqueue.append(make_AV(2, ch))
            pop1()
            if ch >= 2:
                emit_op(ch - 2)
            emit_S(3, ch)
            gqueue.append(make_AV(3, ch))
            pop1()
        emit_op(NG - 2)
        pop1()
        pop1()
        assert not gqueue
        emit_op(NG - 1)
        flush_carry()

        pot_cm.__exit__(None, None, None)
        po_cm.__exit__(None, None, None)
        pss_cm.__exit__(None, None, None)
        pmm_cm.__exit__(None, None, None)
        osp_cm.__exit__(None, None, None)
        lnp_cm.__exit__(None, None, None)
        onp_cm.__exit__(None, None, None)
        etp_cm.__exit__(None, None, None)
        vmp_cm.__exit__(None, None, None)
        gp_cm.__exit__(None, None, None)
        qkp_cm.__exit__(None, None, None)
        dram_cm.__exit__(None, None, None)
        wop_cm.__exit__(None, None, None)
        wp_cm.__exit__(None, None, None)
        xbp_cm.__exit__(None, None, None)
        cp_cm.__exit__(None, None, None)

    _normalize_waits(nc)
    return nc


def _slopes16():
    half = NHEADS // 2
    return np.concatenate(
        [2.0 ** np.linspace(0.0, -8.0, half), np.zeros(NHEADS - half)]
    ).astype(np.float32)


def kernel(x, ln1_g, ln1_b, ln2_g, ln2_b, w_in, w_out, smear_factor, log_scale):
    x = np.asarray(x, np.float32)
    w_in = np.asarray(w_in, np.float32)
    w_out = np.asarray(w_out, np.float32)
    ln1_g = np.asarray(ln1_g, np.float32)
    ln1_b = np.asarray(ln1_b, np.float32)
    ln2_g = np.asarray(ln2_g, np.float32)
    ln2_b = np.asarray(ln2_b, np.float32)
    smear_factor = np.asarray(smear_factor, np.float32)
    log_scale = np.asarray(log_scale, np.float32)

    if "nc" not in _CACHED:
        _CACHED["nc"] = build()
    nc = _CACHED["nc"]

    slopes16 = _slopes16()
    jj = np.arange(128)
    tri = (jj[:, None] <= jj[None, :]).astype(NP_BF16)  # keep j <= i
    iden = np.eye(128, dtype=NP_BF16)
    iota = np.arange(128, dtype=np.float32)

    in_maps = []
    for c in range(8):
        b, r = divmod(c, 4)
        hs = [r, 4 + r, 8 + 2 * r, 9 + 2 * r]
        cols = np.concatenate([np.arange(h * 128, (h + 1) * 128) for h in hs])
        sl = slopes16[hs]
        inv = np.exp(-2.0 * log_scale[hs]) / np.sqrt(128.0)
        sg = 1.0 / (1.0 + np.exp(-smear_factor[hs]))
        om = 1.0 - sg
        ratio = np.exp(smear_factor[hs])

        wq = w_in[:, 0 * DEXP + cols] * ln1_g[:, None]
        wk = w_in[:, 1 * DEXP + cols] * ln1_g[:, None]
        wv = w_in[:, 2 * DEXP + cols] * ln1_g[:, None]
        wp = w_in[:, 3 * DEXP + cols] * ln1_g[:, None]
        cq = ln1_b @ w_in[:, 0 * DEXP + cols]
        ck = ln1_b @ w_in[:, 1 * DEXP + cols]
        cv = ln1_b @ w_in[:, 2 * DEXP + cols]
        cp = ln1_b @ w_in[:, 3 * DEXP + cols]
        for i in range(NH):
            s = slice(i * 128, (i + 1) * 128)
            wq[:, s] *= inv[i]
            wk[:, s] *= om[i]
            cq[s] *= inv[i]
            ck[s] *= om[i]
        ccols = np.stack(
            [cq[i * 128 : (i + 1) * 128] for i in range(NH)]
            + [ck[i * 128 : (i + 1) * 128] for i in range(NH)]
            + [cp[i * 128 : (i + 1) * 128] for i in range(NH)],
            axis=1,
        ).astype(np.float32)
        ratio_t = np.tile(ratio.reshape(1, NH), (128, 1)).astype(np.float32)
        bias_cols = [sl[0] * (iota - 128.0 * d - 63.0) for d in range(3)]
        bias_cols += [sl[1] * (iota - 128.0 * dd - 447.0) for dd in range(-3, 13)]
        biasv = np.stack(bias_cols, axis=1).astype(np.float32)
        smallf = np.concatenate([ccols, ratio_t, biasv], axis=1).astype(np.float32)
        cvbc = np.tile(cv.reshape(1, DL), (128, 1))
        smallb = np.concatenate(
            [tri.astype(np.float32), iden.astype(np.float32), cvbc], axis=1
        ).astype(NP_BF16)
        g2b2 = np.concatenate(
            [np.tile(ln2_g.reshape(1, D), (128, 1)), np.tile(ln2_b.reshape(1, D), (128, 1))],
            axis=1,
        ).astype(NP_BF16)

        m = {
            "xt": np.ascontiguousarray(x[b].T).astype(NP_BF16),
            "wq": np.ascontiguousarray(wq).astype(NP_BF16),
            "wk": np.ascontiguousarray(wk).astype(NP_BF16),
            "wv": np.ascontiguousarray(wv).astype(NP_BF16),
            "wp": np.ascontiguousarray(wp).astype(NP_BF16),
            "wout": np.ascontiguousarray(w_out[cols, :]).astype(NP_BF16),
            "smallf": smallf,
            "smallb": smallb,
            "g2b2": g2b2,
        }
        in_maps.append(m)

    res = None
    last_exc = None
    for _attempt in range(3):
        try:
            res = run_bass_kernel_spmd(nc, in_maps, core_ids=list(range(8)))
            break
        except Exception as e:  # transient axon worker drops; retry
            last_exc = e
            import time as _time

            _time.sleep(2.0)
    if res is None:
        raise last_exc
    _CACHED["last_res"] = res
    out = np.empty((B, L, D), np.float32)
    for c in range(8):
        b, r = divmod(c, 4)
        o = np.asarray(res.results[c]["out"], np.float32)  # [512, 1024]
        for g in range(NG):
            out[b, 512 * g + 128 * r : 512 * g + 128 * r + 128, :] = o[
                128 * g : 128 * (g + 1), :
            ]
    return out
